# revision 7
# baseline (speedup 1.0000x reference)
"""Trainium2 Bass kernel: 3D Gaussian mixture rendered on a voxel grid.

Computes grid[z,y,x] = sum_a amp * prod_axis (voxel-averaged 1D gaussian
integrals via erf), i.e. a sum of 2048 separable outer products.

Strategy:
  - Shard the output grid along y: core i renders y-pixels [16i, 16i+16).
    No collectives; host concatenates the 8 disjoint slabs.
  - Host-side atom culling per slab: atoms farther than MARGIN_SIGMA*sigma
    from the slab contribute < 1e-12 relatively and are dropped. <=640
    atoms survive per slab -> NBLK=5 blocks of 128 atoms (padded; pads are
    zeroed via a mask folded into the per-atom y-weights).
  - Device pipeline (per core):
      ACT:  erf at the 129 pixel *edges* per axis per atom-block (one erf
            per edge; gx[p] = E[p+1]-E[p] reuses each edge eval twice).
      DVE:  batched shifted-slice subtractions over all blocks at once ->
            gx [128a, B, 128], gz [128a, B, 128], gy [128a, B, 16];
            gys = gy * mask (mask carries amp*(0.5/vs)^3, 0 for pads).
      DVE:  Khatri-Rao H[a, y, b, x] = gx[a,b,x] * gys[a,b,y]: one
            broadcast-AP tensor_tensor per y over all blocks (16 ops).
      PE:   grid[z, (y,x)] += gz_b.T @ H_b, accumulated in PSUM over
            blocks (contraction over atoms), fp16 operands at full rate.
      PSUM -> SBUF (DVE/ACT copies) -> HBM.
"""

import os

import numpy as np

import concourse.bacc as bacc
import concourse.tile as tile
from concourse import mybir
from concourse.bass_utils import run_bass_kernel_spmd

N_PIX = 128
N_CORES = 8
SLAB = N_PIX // N_CORES  # 16 y-pixels per core
NBLK = 5  # atom blocks of 128 per core
CAP = NBLK * 128
MARGIN_SIGMA = 7.5  # cull atoms farther than this (in sigmas) from the slab

MM_DT = "f16"  # matmul operand dtype: "f16" | "bf16" | "f32r"

LAST_RESULTS = None  # BassKernelResults of the most recent run (for test.py)

# merged-input column layout
_C_EDGE = 0
_C_YEDGE = _C_EDGE + N_PIX + 1
_C_PX = _C_YEDGE + SLAB + 1
_C_PY = _C_PX + NBLK
_C_PZ = _C_PY + NBLK
_C_MASK = _C_PZ + NBLK
_W_IN = _C_MASK + NBLK


def _build_nc(inv_d: float):
    f32 = mybir.dt.float32
    mm_dt = {
        "f16": mybir.dt.float16,
        "bf16": mybir.dt.bfloat16,
        "f32r": mybir.dt.float32r,
    }[MM_DT]
    Erf = mybir.ActivationFunctionType.Erf
    mult = mybir.AluOpType.mult

    nc = bacc.Bacc(None, target_bir_lowering=False, name="gauss3d")
    inp_d = nc.dram_tensor("inp", [128, _W_IN], f32, kind="ExternalInput")
    grid_d = nc.dram_tensor("grid", [128, SLAB * N_PIX], f32, kind="ExternalOutput")

    with tile.TileContext(nc) as tc:
        with (
            tc.tile_pool(name="const", bufs=1) as const,
            tc.tile_pool(name="work", bufs=2) as work,
            tc.tile_pool(name="o", bufs=2) as opool,
            tc.tile_pool(name="ps", bufs=1, space="PSUM") as psum,
        ):
            inp = const.tile([128, _W_IN], f32)
            nc.sync.dma_start(inp[:], inp_d[:])
            edges = inp[:, _C_EDGE : _C_EDGE + N_PIX + 1]
            yedges = inp[:, _C_YEDGE : _C_YEDGE + SLAB + 1]
            posx = inp[:, _C_PX : _C_PX + NBLK]
            posy = inp[:, _C_PY : _C_PY + NBLK]
            posz = inp[:, _C_PZ : _C_PZ + NBLK]
            mask = inp[:, _C_MASK : _C_MASK + NBLK]

            # activation computes func(in*scale + bias): bias_col = -pos*inv_d
            bx = const.tile([128, NBLK], f32)
            nc.vector.tensor_scalar_mul(bx[:], posx, -inv_d)
            by = const.tile([128, NBLK], f32)
            nc.vector.tensor_scalar_mul(by[:], posy, -inv_d)
            bz = const.tile([128, NBLK], f32)
            nc.vector.tensor_scalar_mul(bz[:], posz, -inv_d)

            ex = work.tile([128, NBLK, N_PIX + 1], f32)
            ez = work.tile([128, NBLK, N_PIX + 1], f32)
            ey = work.tile([128, NBLK, SLAB + 1], f32)
            for b in range(NBLK):
                nc.scalar.activation(ex[:, b, :], edges, Erf, bias=bx[:, b : b + 1], scale=inv_d)
                nc.scalar.activation(ez[:, b, :], edges, Erf, bias=bz[:, b : b + 1], scale=inv_d)
                nc.scalar.activation(ey[:, b, :], yedges, Erf, bias=by[:, b : b + 1], scale=inv_d)

            gx = work.tile([128, NBLK, N_PIX], f32)
            nc.vector.tensor_sub(gx[:], ex[:, :, 1 : N_PIX + 1], ex[:, :, 0:N_PIX])
            gz = work.tile([128, NBLK, N_PIX], mm_dt)
            nc.vector.tensor_sub(gz[:], ez[:, :, 1 : N_PIX + 1], ez[:, :, 0:N_PIX])
            gy = work.tile([128, NBLK, SLAB], f32)
            nc.vector.tensor_sub(gy[:], ey[:, :, 1 : SLAB + 1], ey[:, :, 0:SLAB])
            # mask carries amp*(0.5/vs)^3 for real atoms, 0 for pads
            gys = work.tile([128, NBLK, SLAB], f32)
            nc.vector.tensor_tensor(
                gys[:], gy[:], mask.broadcast_to([128, NBLK, SLAB]), mult
            )

            h = work.tile([128, SLAB, NBLK, N_PIX], mm_dt)
            for y in range(SLAB):
                nc.vector.tensor_tensor(
                    h[:, y, :, :],
                    gx[:],
                    gys[:, :, y].broadcast_to([128, NBLK, N_PIX]),
                    mult,
                )

            ps = psum.tile([128, SLAB * N_PIX], f32)
            for b in range(NBLK):
                for c in range(4):
                    nc.tensor.matmul(
                        ps[:, 512 * c : 512 * (c + 1)],
                        lhsT=gz[:, b, :],
                        rhs=h[:, 4 * c : 4 * c + 4, b, :],
                        start=(b == 0),
                        stop=(b == NBLK - 1),
                        skip_group_check=True,
                    )

            for c in range(4):
                ot = opool.tile([128, 512], f32, tag="ot")
                if c % 2 == 0:
                    nc.vector.tensor_copy(ot[:], ps[:, 512 * c : 512 * (c + 1)])
                else:
                    nc.scalar.copy(ot[:], ps[:, 512 * c : 512 * (c + 1)])
                nc.sync.dma_start(grid_d[:, 512 * c : 512 * (c + 1)], ot[:])

    nc.compile()
    return nc


def _shard_inputs(pos: np.ndarray, sigma: float, vs: float, n_pix: int, c_amp: float):
    """Per-core [128, _W_IN] merged input: edge tiles + culled/padded atoms."""
    edges = ((np.arange(n_pix + 1, dtype=np.float32) - n_pix // 2) - 0.5) * np.float32(vs)

    w = np.float32(MARGIN_SIGMA * sigma)
    in_maps = []
    for i in range(N_CORES):
        e_lo = edges[SLAB * i]
        e_hi = edges[SLAB * i + SLAB]
        py = pos[:, 1]
        m = (py >= e_lo - w) & (py <= e_hi + w)
        idx = np.nonzero(m)[0]
        if len(idx) > CAP:
            # keep the CAP atoms closest to the slab (farther ones are the
            # ones the margin already proved negligible)
            d = np.maximum(0.0, np.maximum(e_lo - py[idx], py[idx] - e_hi))
            idx = idx[np.argsort(d, kind="stable")[:CAP]]
        n = len(idx)
        p = np.zeros((CAP, 3), dtype=np.float32)
        p[:n] = pos[idx]
        # pads: harmless in-range position; mask=0 kills their contribution
        p[n:, 1] = np.float32((e_lo + e_hi) / 2)
        mask = np.zeros((CAP,), dtype=np.float32)
        mask[:n] = np.float32(c_amp)

        def blk(v):  # [CAP] -> [128, NBLK] (partition = index within block)
            return v.reshape(NBLK, 128).T

        buf = np.zeros((128, _W_IN), dtype=np.float32)
        buf[:, _C_EDGE : _C_EDGE + n_pix + 1] = edges[None, :]
        buf[:, _C_YEDGE : _C_YEDGE + SLAB + 1] = edges[None, SLAB * i : SLAB * i + SLAB + 1]
        buf[:, _C_PX : _C_PX + NBLK] = blk(p[:, 0])
        buf[:, _C_PY : _C_PY + NBLK] = blk(p[:, 1])
        buf[:, _C_PZ : _C_PZ + NBLK] = blk(p[:, 2])
        buf[:, _C_MASK : _C_MASK + NBLK] = blk(mask)
        in_maps.append({"inp": buf})
    return in_maps


def kernel(
    atom_positions: np.ndarray,
    log_var: np.ndarray,
    log_weight: np.ndarray,
    n_pix,
    voxel_size,
) -> np.ndarray:
    global LAST_RESULTS
    pos = np.asarray(atom_positions, dtype=np.float32)
    lv = float(np.asarray(log_var, dtype=np.float32).reshape(-1)[0])
    lw = float(np.asarray(log_weight, dtype=np.float32).reshape(-1)[0])
    n_pix = int(n_pix)
    vs = float(voxel_size)
    assert n_pix == N_PIX, f"kernel compiled for n_pix={N_PIX}, got {n_pix}"

    sigma = float(np.exp(0.5 * lv))
    amp = float(np.exp(lw))
    inv_d = float(1.0 / (np.sqrt(2.0) * sigma))
    c_amp = float(amp * (0.5 / vs) ** 3)

    in_maps = _shard_inputs(pos, sigma, vs, n_pix, c_amp)
    nc = _build_nc(inv_d)
    res = run_bass_kernel_spmd(
        nc,
        in_maps,
        core_ids=list(range(N_CORES)),
        trace=bool(int(os.environ.get("GAUSS3D_TRACE", "0"))),
    )
    LAST_RESULTS = res
    grids = [r["grid"].reshape(N_PIX, SLAB, N_PIX) for r in res.results]
    return np.ascontiguousarray(np.concatenate(grids, axis=1), dtype=np.float32)


# revision 8
# speedup vs baseline: 1.1494x; 1.1494x over previous
"""Trainium2 Bass kernel: 3D Gaussian mixture rendered on a voxel grid.

Computes grid[z,y,x] = sum_a amp * prod_axis (voxel-averaged 1D gaussian
integrals via erf), i.e. a sum of 2048 separable outer products.

Strategy:
  - Shard the output grid along y: core i renders y-pixels [16i, 16i+16).
    No collectives; host concatenates the 8 disjoint slabs.
  - Host-side atom culling per slab: only atoms within MARGIN_SIGMA*sigma
    of the slab matter; each core keeps the 512 closest atoms (atoms
    beyond ~5 sigma contribute < 1e-6 relatively) -> NBLK=4 blocks of 128
    (padded; pads are zeroed via a mask folded into the y-weights, and the
    mask also carries the global amp*(0.5/vs)^3 scale).
  - Device pipeline, per 128-atom block (blocks pipeline across engines):
      ACT:  erf at pixel *edges* (one erf per edge; the difference of
            adjacent edge evals gives the voxel-averaged integral). x and
            z edge evals land in one combined tile.
      DVE:  one shifted-slice subtraction for gx|gz (fp16 out), one for
            gy, one mask-mult -> gys; then ONE broadcast-AP tensor_tensor
            builds the Khatri-Rao H[y,x] = gx[x] * gys[y] for all 16 y.
      PE:   grid[z, (y,x)] += gz_b.T @ H_b accumulated in PSUM over
            blocks (contraction over atoms), fp16 operands at full rate.
      PSUM -> SBUF (ScalarE copies) -> HBM.
"""

import os

import numpy as np

import concourse.bacc as bacc
import concourse.bass as bass
import concourse.tile as tile
from concourse import mybir
from concourse.bass_utils import run_bass_kernel_spmd

N_PIX = 128
N_CORES = 8
SLAB = N_PIX // N_CORES  # 16 y-pixels per core
NBLK = 4  # atom blocks of 128 per core
CAP = NBLK * 128
MARGIN_SIGMA = 6.5  # cull atoms farther than this (in sigmas) from the slab

LAST_RESULTS = None  # BassKernelResults of the most recent run (for test.py)

# merged-input column layout
_C_EDGE = 0
_C_YEDGE = _C_EDGE + N_PIX + 1
_C_PX = _C_YEDGE + SLAB + 1
_C_PY = _C_PX + NBLK
_C_PZ = _C_PY + NBLK
_C_MASK = _C_PZ + NBLK
_W_IN = _C_MASK + NBLK

# combined x|z edge-eval tile layout: x erf at [0:129], z erf at [132:261]
_XZ_W = 264
_ZOFF = 132


def _bcast_mid(ap: bass.AP, n: int) -> bass.AP:
    """[128, F] AP -> [128, n, F] with a step-0 middle dim."""
    return bass.AP(
        tensor=ap.tensor, offset=ap.offset, ap=[ap.ap[0], [0, n], *ap.ap[1:]]
    )


def _build_nc(inv_d: float):
    f32 = mybir.dt.float32
    f16 = mybir.dt.float16
    Erf = mybir.ActivationFunctionType.Erf
    mult = mybir.AluOpType.mult

    nc = bacc.Bacc(None, target_bir_lowering=False, name="gauss3d")
    inp_d = nc.dram_tensor("inp", [128, _W_IN], f32, kind="ExternalInput")
    grid_d = nc.dram_tensor("grid", [128, SLAB * N_PIX], f32, kind="ExternalOutput")

    with tile.TileContext(nc) as tc:
        with (
            tc.tile_pool(name="const", bufs=1) as const,
            tc.tile_pool(name="work", bufs=3) as work,
            tc.tile_pool(name="o", bufs=2) as opool,
            tc.tile_pool(name="ps", bufs=1, space="PSUM") as psum,
        ):
            inp = const.tile([128, _W_IN], f32)
            nc.sync.dma_start(inp[:], inp_d[:])
            edges = inp[:, _C_EDGE : _C_EDGE + N_PIX + 1]
            yedges = inp[:, _C_YEDGE : _C_YEDGE + SLAB + 1]
            posx = inp[:, _C_PX : _C_PX + NBLK]
            posy = inp[:, _C_PY : _C_PY + NBLK]
            posz = inp[:, _C_PZ : _C_PZ + NBLK]
            mask = inp[:, _C_MASK : _C_MASK + NBLK]

            # activation computes func(in*scale + bias): bias_col = -pos*inv_d
            bx = const.tile([128, NBLK], f32)
            nc.scalar.mul(bx[:], posx, -inv_d)
            by = const.tile([128, NBLK], f32)
            nc.scalar.mul(by[:], posy, -inv_d)
            bz = const.tile([128, NBLK], f32)
            nc.scalar.mul(bz[:], posz, -inv_d)

            ps = psum.tile([128, SLAB * N_PIX], f32)

            for b in range(NBLK):
                exz = work.tile([128, _XZ_W], f32, tag="exz")
                nc.scalar.activation(
                    exz[:, 0 : N_PIX + 1], edges, Erf, bias=bx[:, b : b + 1], scale=inv_d
                )
                nc.scalar.activation(
                    exz[:, _ZOFF : _ZOFF + N_PIX + 1],
                    edges,
                    Erf,
                    bias=bz[:, b : b + 1],
                    scale=inv_d,
                )
                ey = work.tile([128, SLAB + 1], f32, tag="ey")
                nc.scalar.activation(ey[:], yedges, Erf, bias=by[:, b : b + 1], scale=inv_d)

                # gxz[i] = E[i+1]-E[i]; gx = gxz[0:128], gz = gxz[132:260]
                gxz = work.tile([128, _XZ_W], f16, tag="gxz")
                nc.vector.tensor_sub(
                    gxz[:, 0 : _XZ_W - 4], exz[:, 1 : _XZ_W - 3], exz[:, 0 : _XZ_W - 4]
                )
                gy = work.tile([128, SLAB], f32, tag="gy")
                nc.vector.tensor_sub(gy[:], ey[:, 1 : SLAB + 1], ey[:, 0:SLAB])
                # mask carries amp*(0.5/vs)^3 for real atoms, 0 for pads
                gys = work.tile([128, SLAB], f16, tag="gys")
                nc.vector.tensor_tensor(
                    gys[:], gy[:], mask[:, b : b + 1].broadcast_to([128, SLAB]), mult
                )

                # Khatri-Rao: h[y, x] = gx[x] * gys[y], one op for all 16 y
                h = work.tile([128, SLAB, N_PIX], f16, tag="h")
                nc.vector.tensor_tensor(
                    h[:],
                    _bcast_mid(gxz[:, 0:N_PIX], SLAB),
                    gys[:].broadcast_to([128, SLAB, N_PIX]),
                    mult,
                )

                lhsT = gxz[:, _ZOFF : _ZOFF + N_PIX]
                for c in range(4):
                    nc.tensor.matmul(
                        ps[:, 512 * c : 512 * (c + 1)],
                        lhsT=lhsT,
                        rhs=h[:, 4 * c : 4 * c + 4, :],
                        start=(b == 0),
                        stop=(b == NBLK - 1),
                        skip_group_check=True,
                    )

            for c in range(4):
                ot = opool.tile([128, 512], f32, tag="ot")
                nc.scalar.copy(ot[:], ps[:, 512 * c : 512 * (c + 1)])
                nc.sync.dma_start(grid_d[:, 512 * c : 512 * (c + 1)], ot[:])

    nc.compile()
    return nc


def _shard_inputs(pos: np.ndarray, sigma: float, vs: float, n_pix: int, c_amp: float):
    """Per-core [128, _W_IN] merged input: edge tiles + culled/padded atoms."""
    edges = ((np.arange(n_pix + 1, dtype=np.float32) - n_pix // 2) - 0.5) * np.float32(vs)

    w = np.float32(MARGIN_SIGMA * sigma)
    in_maps = []
    for i in range(N_CORES):
        e_lo = edges[SLAB * i]
        e_hi = edges[SLAB * i + SLAB]
        py = pos[:, 1]
        m = (py >= e_lo - w) & (py <= e_hi + w)
        idx = np.nonzero(m)[0]
        if len(idx) > CAP:
            # keep the CAP atoms closest to the slab; dropped atoms sit
            # beyond ~5 sigma and contribute < 1e-6 relatively
            d = np.maximum(0.0, np.maximum(e_lo - py[idx], py[idx] - e_hi))
            idx = idx[np.argsort(d, kind="stable")[:CAP]]
        n = len(idx)
        p = np.zeros((CAP, 3), dtype=np.float32)
        p[:n] = pos[idx]
        # pads: harmless in-range position; mask=0 kills their contribution
        p[n:, 1] = np.float32((e_lo + e_hi) / 2)
        mask = np.zeros((CAP,), dtype=np.float32)
        mask[:n] = np.float32(c_amp)

        def blk(v):  # [CAP] -> [128, NBLK] (partition = index within block)
            return v.reshape(NBLK, 128).T

        buf = np.zeros((128, _W_IN), dtype=np.float32)
        buf[:, _C_EDGE : _C_EDGE + n_pix + 1] = edges[None, :]
        buf[:, _C_YEDGE : _C_YEDGE + SLAB + 1] = edges[None, SLAB * i : SLAB * i + SLAB + 1]
        buf[:, _C_PX : _C_PX + NBLK] = blk(p[:, 0])
        buf[:, _C_PY : _C_PY + NBLK] = blk(p[:, 1])
        buf[:, _C_PZ : _C_PZ + NBLK] = blk(p[:, 2])
        buf[:, _C_MASK : _C_MASK + NBLK] = blk(mask)
        in_maps.append({"inp": buf})
    return in_maps


def kernel(
    atom_positions: np.ndarray,
    log_var: np.ndarray,
    log_weight: np.ndarray,
    n_pix,
    voxel_size,
) -> np.ndarray:
    global LAST_RESULTS
    pos = np.asarray(atom_positions, dtype=np.float32)
    lv = float(np.asarray(log_var, dtype=np.float32).reshape(-1)[0])
    lw = float(np.asarray(log_weight, dtype=np.float32).reshape(-1)[0])
    n_pix = int(n_pix)
    vs = float(voxel_size)
    assert n_pix == N_PIX, f"kernel compiled for n_pix={N_PIX}, got {n_pix}"

    sigma = float(np.exp(0.5 * lv))
    amp = float(np.exp(lw))
    inv_d = float(1.0 / (np.sqrt(2.0) * sigma))
    c_amp = float(amp * (0.5 / vs) ** 3)

    in_maps = _shard_inputs(pos, sigma, vs, n_pix, c_amp)
    nc = _build_nc(inv_d)
    res = run_bass_kernel_spmd(
        nc,
        in_maps,
        core_ids=list(range(N_CORES)),
        trace=bool(int(os.environ.get("GAUSS3D_TRACE", "0"))),
    )
    LAST_RESULTS = res
    grids = [r["grid"].reshape(N_PIX, SLAB, N_PIX) for r in res.results]
    return np.ascontiguousarray(np.concatenate(grids, axis=1), dtype=np.float32)


# revision 12
# speedup vs baseline: 1.4454x; 1.2574x over previous
"""Trainium2 Bass kernel: 3D Gaussian mixture rendered on a voxel grid.

Computes grid[z,y,x] = sum_a amp * prod_axis (voxel-averaged 1D gaussian
integrals via erf), i.e. a sum of 2048 separable outer products.

Strategy:
  - Shard the output grid along y: core i renders y-pixels [16i, 16i+16).
    No collectives; host concatenates the 8 disjoint slabs.
  - Host-side atom culling per slab: only atoms within MARGIN_SIGMA*sigma
    of the slab matter; each core keeps the 512 closest atoms (atoms
    beyond ~5 sigma contribute < 1e-6 relatively) -> NBLK=4 blocks of 128
    (padded; pads are zeroed via a mask folded into the y-weights, and the
    mask also carries the global amp*(0.5/vs)^3 scale).
  - Device pipeline, per 128-atom block (blocks pipeline across engines):
      ACT:  erf at pixel *edges* (one erf per edge; the difference of
            adjacent edge evals gives the voxel-averaged integral). x and
            z edge evals land in one combined tile.
      DVE:  one shifted-slice subtraction for gx|gz (fp16 out), one for
            gy, one mask-mult -> gys; then ONE broadcast-AP tensor_tensor
            builds the Khatri-Rao H[y,x] = gx[x] * gys[y] for all 16 y.
      PE:   grid[z, (y,x)] += gz_b.T @ H_b accumulated in PSUM over
            blocks (contraction over atoms), fp16 operands at full rate.
      PSUM -> SBUF (ScalarE copies) -> HBM.
"""

import os

import numpy as np

import concourse.bacc as bacc
import concourse.bass as bass
import concourse.tile as tile
from concourse import mybir
from concourse.bass_utils import run_bass_kernel_spmd

N_PIX = 128
N_CORES = 8
SLAB = N_PIX // N_CORES  # 16 y-pixels per core
NBLK = 4  # atom blocks of 128 per core
CAP = NBLK * 128
MARGIN_SIGMA = 6.5  # cull atoms farther than this (in sigmas) from the slab

LAST_RESULTS = None  # BassKernelResults of the most recent run (for test.py)

# merged-input column layout: small control part (pos/mask/yedges) first so
# its DMA lands before the edges part
_C_PX = 0
_C_PY = _C_PX + NBLK
_C_PZ = _C_PY + NBLK
_C_MASK = _C_PZ + NBLK
_C_YEDGE = _C_MASK + NBLK
_C_EDGE = _C_YEDGE + SLAB + 1
_W_CTL = _C_EDGE
_W_IN = _C_EDGE + N_PIX + 1

# combined x|z edge-eval tile layout: x erf at [0:129], z erf at [132:261]
_XZ_W = 264
_ZOFF = 132


def _bcast_mid(ap: bass.AP, n: int) -> bass.AP:
    """[128, F] AP -> [128, n, F] with a step-0 middle dim."""
    return bass.AP(
        tensor=ap.tensor, offset=ap.offset, ap=[ap.ap[0], [0, n], *ap.ap[1:]]
    )


def _build_nc(inv_d: float):
    f32 = mybir.dt.float32
    f16 = mybir.dt.float16
    Erf = mybir.ActivationFunctionType.Erf
    mult = mybir.AluOpType.mult

    nc = bacc.Bacc(None, target_bir_lowering=False, name="gauss3d")
    inp_d = nc.dram_tensor("inp", [128, _W_IN], f32, kind="ExternalInput")
    grid_d = nc.dram_tensor("grid", [128, SLAB * N_PIX], f32, kind="ExternalOutput")

    with tile.TileContext(nc) as tc:
        with (
            tc.tile_pool(name="const", bufs=1) as const,
            tc.tile_pool(name="work", bufs=3) as work,
            tc.tile_pool(name="o", bufs=2) as opool,
            tc.tile_pool(name="ps", bufs=1, space="PSUM") as psum,
        ):
            inp = const.tile([128, _W_IN], f32)
            nc.sync.dma_start(inp[:, 0:_W_CTL], inp_d[:, 0:_W_CTL])
            nc.sync.dma_start(inp[:, _W_CTL:_W_IN], inp_d[:, _W_CTL:_W_IN])
            edges = inp[:, _C_EDGE : _C_EDGE + N_PIX + 1]
            yedges = inp[:, _C_YEDGE : _C_YEDGE + SLAB + 1]
            posx = inp[:, _C_PX : _C_PX + NBLK]
            posy = inp[:, _C_PY : _C_PY + NBLK]
            posz = inp[:, _C_PZ : _C_PZ + NBLK]
            mask = inp[:, _C_MASK : _C_MASK + NBLK]

            # activation computes func(in*scale + bias): bias_col = -pos*inv_d
            bx = const.tile([128, NBLK], f32)
            nc.vector.tensor_scalar_mul(bx[:], posx, -inv_d)
            by = const.tile([128, NBLK], f32)
            nc.vector.tensor_scalar_mul(by[:], posy, -inv_d)
            bz = const.tile([128, NBLK], f32)
            nc.vector.tensor_scalar_mul(bz[:], posz, -inv_d)

            pss = [
                psum.tile([128, 512], f32, tag=f"ps{c}", name=f"ps{c}")
                for c in range(4)
            ]

            for b in range(NBLK):
                exz = work.tile([128, _XZ_W], f32, tag="exz")
                nc.scalar.activation(
                    exz[:, 0 : N_PIX + 1], edges, Erf, bias=bx[:, b : b + 1], scale=inv_d
                )
                nc.scalar.activation(
                    exz[:, _ZOFF : _ZOFF + N_PIX + 1],
                    edges,
                    Erf,
                    bias=bz[:, b : b + 1],
                    scale=inv_d,
                )
                ey = work.tile([128, SLAB + 1], f32, tag="ey")
                nc.scalar.activation(ey[:], yedges, Erf, bias=by[:, b : b + 1], scale=inv_d)

                # gxz[i] = E[i+1]-E[i]; gx = gxz[0:128], gz = gxz[132:260]
                gxz = work.tile([128, _XZ_W], f16, tag="gxz")
                nc.vector.tensor_sub(
                    gxz[:, 0 : _XZ_W - 4], exz[:, 1 : _XZ_W - 3], exz[:, 0 : _XZ_W - 4]
                )
                gy = work.tile([128, SLAB], f32, tag="gy")
                nc.vector.tensor_sub(gy[:], ey[:, 1 : SLAB + 1], ey[:, 0:SLAB])
                # mask carries amp*(0.5/vs)^3 for real atoms, 0 for pads
                gys = work.tile([128, SLAB], f16, tag="gys")
                nc.vector.tensor_tensor(
                    gys[:], gy[:], mask[:, b : b + 1].broadcast_to([128, SLAB]), mult
                )

                # Khatri-Rao: h[y, x] = gx[x] * gys[y], one op for all 16 y
                h = work.tile([128, SLAB, N_PIX], f16, tag="h")
                nc.vector.tensor_tensor(
                    h[:],
                    _bcast_mid(gxz[:, 0:N_PIX], SLAB),
                    gys[:].broadcast_to([128, SLAB, N_PIX]),
                    mult,
                )

                lhsT = gxz[:, _ZOFF : _ZOFF + N_PIX]
                for c in range(4):
                    nc.tensor.matmul(
                        pss[c][:],
                        lhsT=lhsT,
                        rhs=h[:, 4 * c : 4 * c + 4, :],
                        start=(b == 0),
                        stop=(b == NBLK - 1),
                        skip_group_check=True,
                    )
                    if b == NBLK - 1:
                        ot = opool.tile([128, 512], f32, tag=f"ot{c}")
                        if c % 2 == 0:
                            nc.vector.tensor_copy(ot[:], pss[c][:])
                        else:
                            nc.scalar.copy(ot[:], pss[c][:])
                        nc.sync.dma_start(grid_d[:, 512 * c : 512 * (c + 1)], ot[:])

    nc.compile()
    return nc


def _shard_inputs(pos: np.ndarray, sigma: float, vs: float, n_pix: int, c_amp: float):
    """Per-core [128, _W_IN] merged input: edge tiles + culled/padded atoms."""
    edges = ((np.arange(n_pix + 1, dtype=np.float32) - n_pix // 2) - 0.5) * np.float32(vs)

    w = np.float32(MARGIN_SIGMA * sigma)
    in_maps = []
    for i in range(N_CORES):
        e_lo = edges[SLAB * i]
        e_hi = edges[SLAB * i + SLAB]
        py = pos[:, 1]
        m = (py >= e_lo - w) & (py <= e_hi + w)
        idx = np.nonzero(m)[0]
        if len(idx) > CAP:
            # keep the CAP atoms closest to the slab; dropped atoms sit
            # beyond ~5 sigma and contribute < 1e-6 relatively
            d = np.maximum(0.0, np.maximum(e_lo - py[idx], py[idx] - e_hi))
            idx = idx[np.argsort(d, kind="stable")[:CAP]]
        n = len(idx)
        p = np.zeros((CAP, 3), dtype=np.float32)
        p[:n] = pos[idx]
        # pads: harmless in-range position; mask=0 kills their contribution
        p[n:, 1] = np.float32((e_lo + e_hi) / 2)
        mask = np.zeros((CAP,), dtype=np.float32)
        mask[:n] = np.float32(c_amp)

        def blk(v):  # [CAP] -> [128, NBLK] (partition = index within block)
            return v.reshape(NBLK, 128).T

        buf = np.zeros((128, _W_IN), dtype=np.float32)
        buf[:, _C_EDGE : _C_EDGE + n_pix + 1] = edges[None, :]
        buf[:, _C_YEDGE : _C_YEDGE + SLAB + 1] = edges[None, SLAB * i : SLAB * i + SLAB + 1]
        buf[:, _C_PX : _C_PX + NBLK] = blk(p[:, 0])
        buf[:, _C_PY : _C_PY + NBLK] = blk(p[:, 1])
        buf[:, _C_PZ : _C_PZ + NBLK] = blk(p[:, 2])
        buf[:, _C_MASK : _C_MASK + NBLK] = blk(mask)
        in_maps.append({"inp": buf})
    return in_maps


def kernel(
    atom_positions: np.ndarray,
    log_var: np.ndarray,
    log_weight: np.ndarray,
    n_pix,
    voxel_size,
) -> np.ndarray:
    global LAST_RESULTS
    pos = np.asarray(atom_positions, dtype=np.float32)
    lv = float(np.asarray(log_var, dtype=np.float32).reshape(-1)[0])
    lw = float(np.asarray(log_weight, dtype=np.float32).reshape(-1)[0])
    n_pix = int(n_pix)
    vs = float(voxel_size)
    assert n_pix == N_PIX, f"kernel compiled for n_pix={N_PIX}, got {n_pix}"

    sigma = float(np.exp(0.5 * lv))
    amp = float(np.exp(lw))
    inv_d = float(1.0 / (np.sqrt(2.0) * sigma))
    c_amp = float(amp * (0.5 / vs) ** 3)

    in_maps = _shard_inputs(pos, sigma, vs, n_pix, c_amp)
    nc = _build_nc(inv_d)
    res = run_bass_kernel_spmd(
        nc,
        in_maps,
        core_ids=list(range(N_CORES)),
        trace=bool(int(os.environ.get("GAUSS3D_TRACE", "0"))),
    )
    LAST_RESULTS = res
    grids = [r["grid"].reshape(N_PIX, SLAB, N_PIX) for r in res.results]
    return np.ascontiguousarray(np.concatenate(grids, axis=1), dtype=np.float32)
